# revision 2
# baseline (speedup 1.0000x reference)
"""Bass/Trainium2 kernel for nn_BonsaiLayer (soft decision-tree layer).

Data-parallel over 8 NeuronCores (batch axis), software-pipelined (depth 2).
Per core (8192 batch rows, 16 tiles of 512):
  - X arrives host-pretransposed as two planes: Xh fp16 and the residual
    Xl = (X - Xh) * 2048 in fp8-e5m2 (24 MB/core, contiguous per-tile DMA).
  - One fp16 PE pass per tile computes [S*Xp | TZa@Xh | TZb@Xh] (PSUM rows
    0:64 / 64:95 / 96:127); 8 fp8 matmuls accumulate the e5m2(TZ)@Xl
    branch-indicator correction into the TZa rows of the same PSUM bank.
  - th merge: Act copies TZb rows (scale 1/S) to SBUF, DVE adds the TZa+corr
    rows (single-PSUM-operand rule); fp16 PE transpose (32-col-aligned PSUM
    blocks); hard indicators u+/u- are DVE is_ge/is_lt compares; path
    probabilities built level-by-level on Pool (SBUF fp16).
  - Predictor phase per 128-batch j (node dim padded 63->64, 640 cols):
    W/V fp16 matmuls; tanh(V-PSUM) on Act; m = tanh*prob on DVE; W-PSUM
    extraction split cols 0:384 Act-copy + DVE mult, 384:640 DVE direct;
    64->32->16 folds (Pool/DVE) and fp16 reduce -> S-scaled scores.
Engine-legality notes: GPSIMD never touches PSUM; no op reads two PSUM
tensors; PSUM partition offsets 32-aligned; fp16 PSUM writes 4B-aligned.
All shapes hardcoded for X[65536,1024], Z[64,1024], W/V[630,64], T[31,64].
"""
import sys
sys.path.insert(0, '/opt/trn_rl_repo')
import numpy as np
import ml_dtypes
import concourse.bass as bass
import concourse.mybir as mybir
import concourse.tile as tile
from concourse import bacc
from concourse.bass_utils import run_bass_kernel_spmd
from concourse.masks import make_identity

F32, F16 = mybir.dt.float32, mybir.dt.float16
F8E5 = mybir.dt.float8e5
E5 = ml_dtypes.float8_e5m2
AF = mybir.ActivationFunctionType
OP = mybir.AluOpType
DR = mybir.MatmulPerfMode.DoubleRow
AX = mybir.AxisListType

D, P, C, TOT, INT = 1024, 64, 10, 63, 31
NN = 64
CN = C * NN               # 640
NCORES = 8
B = 65536
BC = B // NCORES
NBT = BC // 512
S = 2048.0
O_L = [0, 1, 3, 7, 15, 31]
AW = 384                  # cols extracted via Act copy + DVE mult
DW = 256                  # cols multiplied by DVE directly from PSUM

_ordl = [[0]]
for _ in range(5):
    _ordl.append([2 * n + 1 for n in _ordl[-1]] + [2 * n + 2 for n in _ordl[-1]])
ORDINT = _ordl[0] + _ordl[1] + _ordl[2] + _ordl[3] + _ordl[4]
PERM = ORDINT + _ordl[5]

_nc_cache = None
_last_in_maps = None


def _build_nc(reps=1, loop_reps=None):
    nc = bacc.Bacc(None, target_bir_lowering=False)
    xh_d = nc.dram_tensor("xh", [128, 8, BC], F16, kind="ExternalInput")
    xl_d = nc.dram_tensor("xl", [128, 8, BC], F8E5, kind="ExternalInput")
    l_d = nc.dram_tensor("l", [128, 8 * 128], F16, kind="ExternalInput")
    t8_d = nc.dram_tensor("t8", [128, 8 * 31], F8E5, kind="ExternalInput")
    wv_d = nc.dram_tensor("wv", [64, 2 * CN], F16, kind="ExternalInput")
    out_d = nc.dram_tensor("out", [BC, C], F16, kind="ExternalOutput")

    with tile.TileContext(nc) as tc:
        with tc.tile_pool(name="cst", bufs=1) as cst, \
             tc.tile_pool(name="stage", bufs=4) as stage, \
             tc.tile_pool(name="work", bufs=5) as work, \
             tc.tile_pool(name="work3", bufs=4) as work3, \
             tc.tile_pool(name="mps", bufs=1, space="PSUM") as mps, \
             tc.tile_pool(name="tps", bufs=1, space="PSUM") as tps, \
             tc.tile_pool(name="vps", bufs=1, space="PSUM") as vps, \
             tc.tile_pool(name="wps", bufs=2, space="PSUM") as wps:

            l_sb = cst.tile([128, 8 * 128], F16)
            nc.gpsimd.dma_start(l_sb[:], l_d[:, :])
            t8_sb = cst.tile([128, 8 * 31], F8E5)
            nc.gpsimd.dma_start(t8_sb[:], t8_d[:, :])
            wv_sb = cst.tile([64, 2 * CN], F16)
            nc.gpsimd.dma_start(wv_sb[:], wv_d[:, :])
            ident = cst.tile([INT, INT], F16)
            make_identity(nc, ident[:])
            score_sb = cst.tile([128, NBT * 4 * C], F16)

            import contextlib
            loop_ctx = tc.For_i(0, loop_reps, 1, hint_engines=tuple(nc.engines)) \
                if loop_reps else contextlib.nullcontext()
            with loop_ctx:
             for rep in range(reps):

              def load(t):
                  bs = t * 512
                  xh_t = stage.tile([128, 8, 512], F16, tag="xh")
                  nc.sync.dma_start(xh_t[:], xh_d[:, :, bs:bs + 512])
                  xl_t = stage.tile([128, 8, 512], F8E5, tag="xl")
                  nc.sync.dma_start(xl_t[:], xl_d[:, :, bs:bs + 512])
                  return xh_t, xl_t

              def jiter(prev, j):
                  """Predictor j-iteration for the carried tile."""
                  t0, p_xph, p_prb = prev
                  tt = t0 * 4 + j
                  lhsT = p_xph[:, j * 128:(j + 1) * 128]
                  wvpV = vps.tile([128, CN], F32)
                  nc.tensor.matmul(wvpV[:, 0:512], lhsT, wv_sb[:, CN:CN + 512])
                  nc.tensor.matmul(wvpV[:, 512:CN], lhsT,
                                   wv_sb[:, CN + 512:2 * CN])
                  tnh = work3.tile([128, CN], F16, tag="tnh")
                  nc.scalar.activation(tnh[:], wvpV[:], AF.Tanh, scale=1.0 / S)
                  m_t = work3.tile([128, CN], F16, tag="m")
                  pb = p_prb[:, j * NN:(j + 1) * NN].unsqueeze(1) \
                      .broadcast_to((128, C, NN))
                  nc.vector.tensor_tensor(
                      m_t[:].rearrange("p (c q) -> p c q", c=C),
                      tnh[:].rearrange("p (c q) -> p c q", c=C), pb, OP.mult)
                  wvpW = wps.tile([128, CN], F32)
                  nc.tensor.matmul(wvpW[:, 0:512], lhsT, wv_sb[:, 0:512])
                  nc.tensor.matmul(wvpW[:, 512:CN], lhsT, wv_sb[:, 512:CN])
                  wx9 = work3.tile([128, AW], F16, tag="wx9")
                  nc.scalar.copy(wx9[:], wvpW[:, 0:AW])
                  h_t = work3.tile([128, CN], F16, tag="h")
                  nc.vector.tensor_tensor(h_t[:, 0:AW], wx9[:],
                                          m_t[:, 0:AW], OP.mult)
                  nc.vector.tensor_tensor(h_t[:, AW:CN], wvpW[:, AW:CN],
                                          m_t[:, AW:CN], OP.mult)
                  h3 = h_t[:].rearrange("p (c q) -> p c q", c=C)
                  f_t = work3.tile([128, C * 32], F16, tag="f")
                  f3 = f_t[:].rearrange("p (c q) -> p c q", c=C)
                  nc.gpsimd.tensor_tensor(f3[:], h3[:, :, 0:32],
                                          h3[:, :, 32:64], OP.add)
                  g_t = work3.tile([128, C * 16], F16, tag="g2")
                  g3 = g_t[:].rearrange("p (c q) -> p c q", c=C)
                  eng2 = nc.gpsimd if j % 2 == 0 else nc.vector
                  eng2.tensor_tensor(g3[:], f3[:, :, 0:16],
                                     f3[:, :, 16:32], OP.add)
                  with nc.allow_low_precision(reason="fp16 score accumulate"):
                      nc.vector.tensor_reduce(
                          score_sb[:, tt * C:(tt + 1) * C], g3,
                          axis=AX.X, op=OP.add)

              pend = []
              carried = []
              for t in range(NBT + 2):
                  if t == 0:
                      pend.append(load(0))
                      pend.append(load(1))
                  elif t + 1 < NBT:
                      pend.append(load(t + 1))

                  prev = carried.pop(0) if len(carried) == 2 or t >= NBT \
                      else None
                  cur = None
                  if t < NBT:
                      cur = pend.pop(0)

                  # interleave: proj k0-3 | j0 | proj k4-7 + corr + th/xph |
                  #             j1 | transpose/cmp | j2 | tree | j3
                  if cur is not None:
                      xh_t, xl_t = cur
                      psm = mps.tile([128, 512], F32)
                      for k in range(4):
                          nc.tensor.matmul(psm[:, :],
                                           l_sb[:, k * 128:(k + 1) * 128],
                                           xh_t[:, k, :],
                                           start=(k == 0), stop=False)
                  if prev is not None:
                      jiter(prev, 0)
                  if cur is not None:
                      for k in range(4, 8):
                          nc.tensor.matmul(psm[:, :],
                                           l_sb[:, k * 128:(k + 1) * 128],
                                           xh_t[:, k, :],
                                           start=False, stop=False)
                      for q in range(8):
                          nc.tensor.matmul(psm[64:95, :],
                                           t8_sb[:, q * 31:(q + 1) * 31],
                                           xl_t[:, q, :],
                                           start=False, stop=(q == 7))
                      bcp = work.tile([INT, 512], F16, tag="bcp")
                      nc.scalar.activation(bcp[:], psm[96:127, :], AF.Copy,
                                           scale=1.0 / S)
                      th16 = work.tile([INT, 512], F16, tag="th")
                      nc.vector.tensor_tensor(th16[:], bcp[:],
                                              psm[64:95, :], OP.add)
                      xph2 = work.tile([64, 512], F16, tag="xph")
                      nc.scalar.copy(xph2[:], psm[0:64, :])
                  if prev is not None:
                      jiter(prev, 1)
                  if cur is not None:
                      thT = tps.tile([128, 4 * 32], F16)
                      for j in range(4):
                          nc.tensor.transpose(thT[:, j * 32:j * 32 + INT],
                                              th16[:, j * 128:(j + 1) * 128],
                                              ident[:])
                      thT3 = thT[:].rearrange("p (j n) -> p j n", j=4)[:, :, 0:INT]
                      upm = work.tile([128, 248], F16, tag="upm")
                      nc.vector.tensor_scalar(
                          upm[:, 0:124].rearrange("p (j n) -> p j n", j=4),
                          thT3, 0.0, None, OP.is_ge)
                      nc.vector.tensor_scalar(
                          upm[:, 124:248].rearrange("p (j n) -> p j n", j=4),
                          thT3, 0.0, None, OP.is_lt)
                  if prev is not None:
                      jiter(prev, 2)
                  if cur is not None:
                      prb = work.tile([128, 4 * NN], F16, tag="prb")
                      p3 = prb[:].rearrange("p (j n) -> p j n", j=4)
                      nc.gpsimd.memset(p3[:, :, 0:1], 1.0)
                      nc.gpsimd.memset(p3[:, :, 63:64], 0.0)
                      u4 = upm[:].rearrange("p (s j n) -> p j s n", s=2, j=4)
                      for l in range(1, 6):
                          h = 2 ** (l - 1)
                          out_ap = p3[:, :, O_L[l]:O_L[l] + 2 * h].rearrange(
                              "p j (s i) -> p j s i", s=2)
                          in0 = p3[:, :, O_L[l - 1]:O_L[l - 1] + h] \
                              .unsqueeze(2).broadcast_to((128, 4, 2, h))
                          in1 = u4[:, :, :, O_L[l - 1]:O_L[l - 1] + h]
                          nc.gpsimd.tensor_tensor(out_ap, in0, in1, OP.mult)
                      carried.append((t, xph2, prb))
                  if prev is not None:
                      jiter(prev, 3)

            nc.sync.dma_start(out_d.rearrange("(t p) c -> p t c", p=128),
                              score_sb[:].rearrange("p (t c) -> p t c", c=C))
    nc.finalize()
    return nc


def _get_nc():
    global _nc_cache
    if _nc_cache is None:
        _nc_cache = _build_nc()
    return _nc_cache


def _host_prep(X, Z, W, V, T):
    X = np.ascontiguousarray(np.asarray(X, dtype=np.float32))
    Z = np.asarray(Z, dtype=np.float64)
    W = np.asarray(W, dtype=np.float64)
    V = np.asarray(V, dtype=np.float64)
    T = np.asarray(T, dtype=np.float64)

    Zs = Z / P
    TZ = T[ORDINT] @ Zs
    TZa = (TZ * S).astype(np.float16)
    TZb = ((TZ * S - TZa.astype(np.float64)) * S).astype(np.float16)
    L = np.zeros((128, 8 * 128), np.float16)
    ZsT = (Zs * S).astype(np.float16).T
    for c in range(8):
        r = slice(c * 128, (c + 1) * 128)
        L[:, c * 128 + 0:c * 128 + 64] = ZsT[r]
        L[:, c * 128 + 64:c * 128 + 95] = TZa.T[r]
        L[:, c * 128 + 96:c * 128 + 128] = np.concatenate(
            [TZb.T[r], np.zeros((128, 1), np.float16)], axis=1)
    T8 = np.zeros((128, 8 * 31), E5)
    TZ8T = TZ.astype(E5).T
    for c in range(8):
        T8[:, c * 31:(c + 1) * 31] = TZ8T[c * 128:(c + 1) * 128]

    W3 = W.reshape(TOT, C, P)
    V3 = V.reshape(TOT, C, P)
    Wt = np.zeros((P, CN), np.float16)
    Vt = np.zeros((P, CN), np.float16)
    Wt.reshape(P, C, NN)[:, :, 0:TOT] = \
        W3[PERM].transpose(2, 1, 0).astype(np.float16)
    Vt.reshape(P, C, NN)[:, :, 0:TOT] = \
        V3[PERM].transpose(2, 1, 0).astype(np.float16)
    WVt = np.concatenate([Wt, Vt], axis=1)

    Xh = X.astype(np.float16)
    Xl8 = ((X - Xh.astype(np.float32)) * np.float32(S)).astype(E5)

    in_maps = []
    for c in range(NCORES):
        sl = slice(c * BC, (c + 1) * BC)
        xh_c = np.ascontiguousarray(
            Xh[sl].T.reshape(8, 128, BC).transpose(1, 0, 2))
        xl_c = np.ascontiguousarray(
            Xl8[sl].T.reshape(8, 128, BC).transpose(1, 0, 2))
        in_maps.append({"xh": xh_c, "xl": xl_c, "l": L, "t8": T8, "wv": WVt})
    return in_maps


def kernel(X, Z, W, V, T):
    global _last_in_maps
    in_maps = _host_prep(X, Z, W, V, T)
    _last_in_maps = in_maps
    nc = _get_nc()
    res = run_bass_kernel_spmd(nc, in_maps, core_ids=list(range(NCORES)))
    score = np.concatenate([r["out"] for r in res.results], axis=0)
    return np.ascontiguousarray(
        (score.astype(np.float32).T * np.float32(1.0 / S)))
